# revision 20
# baseline (speedup 1.0000x reference)
"""Trainium2 Bass kernel for LocalSquaredDistanceLayer (shapelet min-distance).

Math (matches reference):
  out[b,t,k'] = min_s || xn[b, t+8j : t+8j+8, ch] - kern_n[s, k', :] ||^2
  with k' = 4*ch + j, xn z-normalized per (b,ch) over time, kern_n
  z-normalized per shapelet over (KSZ, C).

One fp16 matmul per (ch, t-chunk) produces all 64 shapelet distances for
32 output columns; a DVE min-reduce finishes each 128-row chunk.

Precision: every term derives from the SAME fp16-quantized x-hat / k-hat,
with exact fp16->fp32 products in psum, so psum is exactly
||x-hat - k-hat||^2: the large-magnitude carriers (window power P = sum x^2,
and ||k||^2) are encoded as fp16 hi+lo row pairs computed in fp32.  Error is
~2*sqrt(D)*||quant|| ~ 1e-3 relative even at the smallest minima (~0.11),
safely under the 2e-2 gate.

Per-core layout (2 batches/core, kernel replicated; 8 cores data-parallel):
  Hbig [42, 16*512] fp16, one 512-col block per signal (b,ch):
    rows 0-31: xn-hat shifted 0..31
    rows 32-35: P_hi[t+8j], rows 36-39: P_lo[t+8j]   (window power hi/lo)
    rows 40-41: ones (K2 hi/lo carriers)
  F_all [42, 2048] fp16, col(ch,j,s) = ch*256 + j*64 + s:
    rows 8j+c: -2*k-hat taps; rows 32+j / 36+j: delta_j ones; 40/41: K2 hi/lo
  Main loop per (b, t-chunk): 8 matmuls -> psum [128, 2048], DVE min-reduce
  over innermost 64 shapelets, DMA out on the gpsimd queue.
"""

import sys

for _p in ("/opt/trn_rl_repo",):
    if _p not in sys.path:
        sys.path.insert(0, _p)

import numpy as np

B, T, C = 16, 512, 8
S, KSZ = 64, 32
TOUT = T - KSZ + 1  # 481
NCORES = 8
BPC = B // NCORES  # batches per core
NSIG = BPC * C  # signals per core
EPS = 1e-8
SIGW = 544  # staged fp16 signal row: xn(512) + zero pad(32)

_cache = {}


def _rap(base, dims):
    """Raw AP at base slice's offset with explicit [step, count] dims (elems)."""
    import concourse.bass as bass

    return bass.AP(tensor=base.tensor, offset=base.offset, ap=[list(d) for d in dims])


def _build_nc():
    import concourse.bass as bass
    import concourse.bacc as bacc
    import concourse.tile as tile
    from concourse import mybir
    from concourse.masks import make_identity
    from contextlib import ExitStack

    f32 = mybir.dt.float32
    f16 = mybir.dt.float16
    AX = mybir.AxisListType
    OP = mybir.AluOpType
    ACT = mybir.ActivationFunctionType

    nc = bacc.Bacc("TRN2", target_bir_lowering=False, debug=False)
    x_d = nc.dram_tensor("x", [BPC, T, C], f32, kind="ExternalInput").ap()
    k_d = nc.dram_tensor("kernel", [S, KSZ, C], f32, kind="ExternalInput").ap()
    o_d = nc.dram_tensor("out", [BPC, TOUT, KSZ], f32, kind="ExternalOutput").ap()

    with tile.TileContext(nc) as tc, ExitStack() as ctx:
        const = ctx.enter_context(tc.tile_pool(name="const", bufs=1))
        outp = ctx.enter_context(tc.tile_pool(name="outp", bufs=3))
        dram = ctx.enter_context(tc.tile_pool(name="dram", bufs=1, space="DRAM"))

        F_all = const.tile([42, 2048], f16, tag="F_all")
        Hbig = const.tile([42, NSIG * T], f16, tag="Hbig")
        Xnb = const.tile([NSIG, SIGW], f16, tag="Xnb")

        XnD = dram.tile([NSIG, SIGW], f16, tag="XnD")
        OnD = dram.tile([NSIG, 1024], f16, tag="OnD")
        PexpD = dram.tile([NSIG, 4096], f16, tag="PexpD")
        FxD = dram.tile([128, 128], f16, tag="FxD")
        K2D = dram.tile([2, 2048], f16, tag="K2D")

        with tc.tile_pool(name="pprep", bufs=1, space="PSUM") as pprep, \
             tc.tile_pool(name="ldp", bufs=2) as ldp:
            # activation-table preload so the ~1.3us load overlaps input DMAs
            dm0 = ldp.tile([1, 1], f32, tag="dm0")
            nc.vector.memset(dm0[:], 1.0)
            dm1 = ldp.tile([1, 1], f32, tag="dm1")
            nc.scalar.activation(out=dm1[:], in_=dm0[:], func=ACT.Sqrt)

            # input loads first: x on sync, kernel on sync
            X0 = ldp.tile([128, BPC * 32], f32, tag="X0")
            for b in range(BPC):
                nc.sync.dma_start(
                    out=X0[:, b * 32:(b + 1) * 32],
                    in_=_rap(x_d[b:b + 1, 0:1, 0:1],
                             [[8, 128], [1024, 4], [1, 8]]))
            KN = ldp.tile([S, KSZ * C], f32, tag="KN")
            nc.sync.dma_start(out=KN[:], in_=k_d.rearrange("s k c -> s (k c)"))

            ident = const.tile([128, 128], f32, tag="ident")
            make_identity(nc, ident[:])
            identh = const.tile([S, S], f16, tag="identh")
            make_identity(nc, identh[:])
            onesW = ldp.tile([NSIG, 1024], f16, tag="onesW")
            nc.gpsimd.memset(onesW[:], 1.0)
            nc.gpsimd.dma_start(out=OnD[:], in_=onesW[:])

            # ---------- shapelet-kernel chain ----------
            kst = ldp.tile([S, nc.vector.BN_STATS_DIM], f32, tag="kst")
            nc.vector.bn_stats(out=kst[:], in_=KN[:])
            mvk = ldp.tile([S, nc.vector.BN_AGGR_DIM], f32, tag="mvk")
            nc.vector.bn_aggr(out=mvk[:], in_=kst[:])
            kstd = ldp.tile([S, 1], f32, tag="kstd")
            nc.scalar.activation(out=kstd[:], in_=mvk[:, 1:2], func=ACT.Sqrt)
            krstd = ldp.tile([S, 1], f32, tag="krstd")
            nc.vector.reciprocal(out=krstd[:], in_=kstd[:])
            kscale = ldp.tile([S, 1], f32, tag="kscale")
            nc.vector.tensor_scalar_mul(kscale[:], krstd[:], -2.0)
            kbias = ldp.tile([S, 1], f32, tag="kbias")
            nc.vector.scalar_tensor_tensor(
                out=kbias[:], in0=mvk[:, 0:1], scalar=2.0, in1=krstd[:],
                op0=OP.mult, op1=OP.mult)
            KNmh = ldp.tile([S, KSZ * C], f16, tag="KNmh")
            nc.vector.tensor_scalar(
                out=KNmh[:], in0=KN[:], scalar1=kscale[:], scalar2=kbias[:],
                op0=OP.mult, op1=OP.add)
            # K2 = 0.25 * sum_c KNmh^2 in fp32 (exact squares of fp16 taps)
            KN2 = ldp.tile([S, KSZ * C], f32, tag="KN2")
            nc.vector.tensor_tensor(
                out=KN2[:], in0=KNmh[:], in1=KNmh[:], op=OP.mult)
            K2sn = ldp.tile([S, KSZ], f32, tag="K2sn")
            nc.vector.tensor_reduce(
                out=K2sn[:], in_=KN2[:].rearrange("s (k c) -> s k c", c=C),
                axis=AX.X, op=OP.add)
            nc.vector.tensor_scalar_mul(K2sn[:], K2sn[:], 0.25)
            K2hi = ldp.tile([S, KSZ], f16, tag="K2hi")
            nc.scalar.copy(out=K2hi[:], in_=K2sn[:])
            K2hi32 = ldp.tile([S, KSZ], f32, tag="K2hi32")
            nc.scalar.copy(out=K2hi32[:], in_=K2hi[:])
            K2lo = ldp.tile([S, KSZ], f16, tag="K2lo")
            nc.vector.tensor_tensor(
                out=K2lo[:], in0=K2sn[:], in1=K2hi32[:], op=OP.subtract)

            # transposes + staging of taps / K2
            TPp = pprep.tile([128, 128], f16, tag="TPp")
            nc.tensor.transpose(TPp[:, 0:64], KNmh[:, 0:128], identh[:, :])
            nc.tensor.transpose(TPp[:, 64:128], KNmh[:, 128:256], identh[:, :])
            K2Tph = pprep.tile([KSZ, S], f16, tag="K2Tph")
            nc.tensor.transpose(K2Tph[:], K2hi[:], identh[:, :])
            K2Tpl = pprep.tile([KSZ, S], f16, tag="K2Tpl")
            nc.tensor.transpose(K2Tpl[:], K2lo[:], identh[:, :])
            TPsb = ldp.tile([128, 128], f16, tag="TPsb")
            nc.scalar.copy(out=TPsb[:], in_=TPp[:])
            K2sb = ldp.tile([KSZ, 128], f16, tag="K2sb")
            nc.scalar.copy(out=K2sb[:, 0:64], in_=K2Tph[:])
            nc.scalar.copy(out=K2sb[:, 64:128], in_=K2Tpl[:])
            # FxD[kp*512 + c*64 + s] = KNmh[s, kp*8+c]
            nc.scalar.dma_start(
                out=_rap(FxD[0:1, 0:1], [[512, 16], [64, 8], [1, 64]]),
                in_=TPsb[:, 0:64])
            nc.scalar.dma_start(
                out=_rap(FxD[64:65, 0:1], [[512, 16], [64, 8], [1, 64]]),
                in_=TPsb[:, 64:128])
            nc.scalar.dma_start(
                out=_rap(K2D[0:1, 0:1], [[64, 32], [2048, 2], [1, 64]]),
                in_=_rap(K2sb[0:1, 0:1], [[128, 32], [64, 2], [1, 64]]))

            # ---------- F_all scatter ----------
            nc.gpsimd.memset(F_all[:], 0.0)
            for j in range(4):
                nc.sync.dma_start(
                    out=_rap(F_all[8 * j:8 * j + 8, 64 * j:64 * j + 1],
                             [[2048, 8], [256, 8], [1, 64]]),
                    in_=_rap(FxD[4 * j:4 * j + 1, 0:1],
                             [[64, 8], [2048, 8], [1, 64]]))
            # delta_j window-power taps: rows 32+j (hi) / 36+j (lo), block j
            for base in (32, 36):
                for j in range(4):
                    eng = nc.scalar if base == 32 else nc.gpsimd
                    eng.dma_start(
                        out=_rap(F_all[base + j:base + j + 1, 64 * j:64 * j + 1],
                                 [[2048, 1], [256, 8], [1, 64]]),
                        in_=_rap(OnD[0:1, 0:1], [[64, 8], [1, 64]]))
            nc.scalar.dma_start(out=F_all[40:42, :], in_=K2D[:])

            # ---------- x chain ----------
            PXp = pprep.tile([8, BPC * T], f32, tag="PXp")
            for b in range(BPC):
                for cc in range(4):
                    nc.tensor.transpose(
                        PXp[:, (b * 4 + cc) * 128:(b * 4 + cc + 1) * 128],
                        X0[:, (b * 4 + cc) * 8:(b * 4 + cc + 1) * 8],
                        ident[:, :])
            Xs8 = ldp.tile([8, BPC * T], f32, tag="Xs8")
            nc.scalar.copy(out=Xs8[:], in_=PXp[:])
            Xsig = ldp.tile([NSIG, T], f32, tag="Xsig")
            for b in range(BPC):
                nc.scalar.dma_start(
                    out=Xsig[b * C:(b + 1) * C, :],
                    in_=Xs8[:, b * T:(b + 1) * T])

            xst = ldp.tile([NSIG, nc.vector.BN_STATS_DIM], f32, tag="xst")
            nc.vector.bn_stats(out=xst[:], in_=Xsig[:])
            mvx = ldp.tile([NSIG, nc.vector.BN_AGGR_DIM], f32, tag="mvx")
            nc.vector.bn_aggr(out=mvx[:], in_=xst[:])
            xstd = ldp.tile([NSIG, 1], f32, tag="xstd")
            nc.scalar.activation(out=xstd[:], in_=mvx[:, 1:2], func=ACT.Sqrt)
            xrstd = ldp.tile([NSIG, 1], f32, tag="xrstd")
            nc.vector.reciprocal(out=xrstd[:], in_=xstd[:])
            xbias = ldp.tile([NSIG, 1], f32, tag="xbias")
            nc.vector.scalar_tensor_tensor(
                out=xbias[:], in0=mvx[:, 0:1], scalar=-1.0, in1=xrstd[:],
                op0=OP.mult, op1=OP.mult)

            # x-hat (fp16) + zero pad; stage for the Hankel loads
            nc.vector.memset(Xnb[:, 512:SIGW], 0.0)
            nc.vector.tensor_scalar(
                out=Xnb[:, 0:T], in0=Xsig[:], scalar1=xrstd[:], scalar2=xbias[:],
                op0=OP.mult, op1=OP.add)
            nc.sync.dma_start(
                out=_rap(XnD[0:1, 0:1], [[SIGW, NSIG], [1, SIGW]]),
                in_=Xnb[:, :])

            # ---------- P-hat path ----------
            # x2f = x-hat^2 (exact fp32), sliding window-8 sums
            x2f = ldp.tile([NSIG, SIGW], f32, tag="x2f")
            nc.vector.tensor_tensor(
                out=x2f[:], in0=Xnb[:, :], in1=Xnb[:, :], op=OP.mult)
            Pw1 = ldp.tile([NSIG, SIGW], f32, tag="Pw1")
            nc.vector.tensor_tensor(
                out=Pw1[:, 0:543], in0=x2f[:, 0:543], in1=x2f[:, 1:544],
                op=OP.add)
            Pw2 = ldp.tile([NSIG, SIGW], f32, tag="Pw2")
            nc.vector.tensor_tensor(
                out=Pw2[:, 0:541], in0=Pw1[:, 0:541], in1=Pw1[:, 2:543],
                op=OP.add)
            Pw = ldp.tile([NSIG, SIGW], f32, tag="Pw")
            nc.vector.memset(Pw[:, 536:SIGW], 0.0)
            nc.vector.tensor_tensor(
                out=Pw[:, 0:537], in0=Pw2[:, 0:537], in1=Pw2[:, 4:541],
                op=OP.add)
            # split P into fp16 hi + lo
            Phi = ldp.tile([NSIG, SIGW], f16, tag="Phi")
            nc.scalar.copy(out=Phi[:], in_=Pw[:])
            Phi32 = ldp.tile([NSIG, SIGW], f32, tag="Phi32")
            nc.scalar.copy(out=Phi32[:], in_=Phi[:])
            Plo = ldp.tile([NSIG, SIGW], f16, tag="Plo")
            nc.vector.tensor_tensor(
                out=Plo[:], in0=Pw[:], in1=Phi32[:], op=OP.subtract)
            # PwHL[sig, (h2, j, t)] = P_{hi/lo}[sig][8j + t]
            PwHL = ldp.tile([NSIG, 4096], f16, tag="PwHL")
            nc.sync.dma_start(
                out=PwHL[:, 0:2048],
                in_=_rap(Phi[0:1, 0:1], [[SIGW, NSIG], [8, 4], [1, 512]]))
            nc.sync.dma_start(
                out=PwHL[:, 2048:4096],
                in_=_rap(Plo[0:1, 0:1], [[SIGW, NSIG], [8, 4], [1, 512]]))
            nc.sync.dma_start(
                out=_rap(PexpD[0:1, 0:1], [[4096, NSIG], [1, 4096]]),
                in_=PwHL[:, :])

            # ---------- Hankel loads ----------
            # P rows 32..39 for all signals in one DMA
            nc.sync.dma_start(
                out=Hbig[32:40, :],
                in_=_rap(PexpD[0:1, 0:1], [[512, 8], [4096, NSIG], [1, 512]]))
            # ones rows 40..41
            nc.gpsimd.dma_start(
                out=Hbig[40:42, :],
                in_=_rap(OnD[0:1, 0:1], [[8192, 2], [1, 8192]]))
            # xn rows 0..31, two signals per DMA
            for i in range(8):
                eng = nc.sync if i % 2 == 0 else nc.scalar
                eng.dma_start(
                    out=_rap(Hbig[0:1, 2 * i * T:2 * i * T + 1],
                             [[NSIG * T, KSZ], [T, 2], [1, T]]),
                    in_=_rap(XnD[2 * i:2 * i + 1, 0:1],
                             [[1, KSZ], [SIGW, 2], [1, T]]))

        # ---------- main loop: matmuls + min + store ----------
        with tc.tile_pool(name="pmm", bufs=2, space="PSUM") as pmm:
            for b in range(BPC):
                for cc in range(4):
                    c0 = cc * 128
                    cnt = 128 if cc < 3 else TOUT - 3 * 128
                    acc = pmm.tile([128, 2048], f32, tag="acc")
                    for ch in range(C):
                        sig = b * C + ch
                        nc.tensor.matmul(
                            acc[:, ch * 256:(ch + 1) * 256],
                            lhsT=Hbig[0:42, sig * T + c0:sig * T + c0 + 128],
                            rhs=F_all[0:42, ch * 256:(ch + 1) * 256],
                            start=True, stop=True)
                    PM = outp.tile([128, KSZ], f32, tag="PM")
                    nc.vector.tensor_reduce(
                        out=PM[:],
                        in_=acc[:].rearrange("p (g s) -> p g s", s=S),
                        axis=AX.X, op=OP.min)
                    nc.gpsimd.dma_start(out=o_d[b, c0:c0 + cnt, :],
                                        in_=PM[0:cnt, :])

    nc.compile()
    return nc


def get_nc():
    if "nc" not in _cache:
        _cache["nc"] = _build_nc()
    return _cache["nc"]


def kernel(x: np.ndarray, kernel: np.ndarray) -> np.ndarray:
    from concourse.bass_utils import run_bass_kernel_spmd

    nc = get_nc()
    x = np.ascontiguousarray(x, dtype=np.float32)
    kern = np.ascontiguousarray(kernel, dtype=np.float32)
    in_maps = [
        {"x": x[i * BPC:(i + 1) * BPC], "kernel": kern} for i in range(NCORES)
    ]
    res = run_bass_kernel_spmd(nc, in_maps, core_ids=list(range(NCORES)))
    return np.concatenate([r["out"] for r in res.results], axis=0)


if __name__ == "__main__":
    rng = np.random.default_rng(0)
    x = rng.standard_normal((B, T, C), dtype=np.float32)
    k = rng.uniform(-0.05, 0.05, (S, KSZ, C)).astype(np.float32)
    out = kernel(x=x, kernel=k)
    print(out.shape, out.dtype)


# revision 21
# speedup vs baseline: 1.0832x; 1.0832x over previous
"""Trainium2 Bass kernel for LocalSquaredDistanceLayer (shapelet min-distance).

Math (matches reference):
  out[b,t,k'] = min_s || xn[b, t+8j : t+8j+8, ch] - kern_n[s, k', :] ||^2
  with k' = 4*ch + j, xn z-normalized per (b,ch) over time, kern_n
  z-normalized per shapelet over (KSZ, C).

One fp16 matmul per (ch, t-chunk) produces all 64 shapelet distances for
32 output columns; a DVE min-reduce finishes each 128-row chunk.

Precision: every term derives from the SAME fp16-quantized x-hat / k-hat,
with exact fp16->fp32 products in psum, so psum is exactly
||x-hat - k-hat||^2: the large-magnitude carriers (window power P = sum x^2,
and ||k||^2) are encoded as fp16 hi+lo row pairs computed in fp32.  Error is
~2*sqrt(D)*||quant|| ~ 1e-3 relative even at the smallest minima (~0.11),
safely under the 2e-2 gate.

Per-core layout (2 batches/core, kernel replicated; 8 cores data-parallel):
  Hbig [42, 16*512] fp16, one 512-col block per signal (b,ch):
    rows 0-31: xn-hat shifted 0..31
    rows 32-35: P_hi[t+8j], rows 36-39: P_lo[t+8j]   (window power hi/lo)
    rows 40-41: ones (K2 hi/lo carriers)
  F_all [42, 2048] fp16, col(ch,j,s) = ch*256 + j*64 + s:
    rows 8j+c: -2*k-hat taps; rows 32+j / 36+j: delta_j ones; 40/41: K2 hi/lo
  Main loop per (b, t-chunk): 8 matmuls -> psum [128, 2048], DVE min-reduce
  over innermost 64 shapelets, DMA out on the gpsimd queue.
"""

import sys

for _p in ("/opt/trn_rl_repo",):
    if _p not in sys.path:
        sys.path.insert(0, _p)

import numpy as np

B, T, C = 16, 512, 8
S, KSZ = 64, 32
TOUT = T - KSZ + 1  # 481
NCORES = 8
BPC = B // NCORES  # batches per core
NSIG = BPC * C  # signals per core
EPS = 1e-8
SIGW = 544  # staged fp16 signal row: xn(512) + zero pad(32)

_cache = {}


def _rap(base, dims):
    """Raw AP at base slice's offset with explicit [step, count] dims (elems)."""
    import concourse.bass as bass

    return bass.AP(tensor=base.tensor, offset=base.offset, ap=[list(d) for d in dims])


def _build_nc():
    import concourse.bass as bass
    import concourse.bacc as bacc
    import concourse.tile as tile
    from concourse import mybir
    from concourse.masks import make_identity
    from contextlib import ExitStack

    f32 = mybir.dt.float32
    f16 = mybir.dt.float16
    AX = mybir.AxisListType
    OP = mybir.AluOpType
    ACT = mybir.ActivationFunctionType

    nc = bacc.Bacc("TRN2", target_bir_lowering=False, debug=False)
    x_d = nc.dram_tensor("x", [BPC, T, C], f32, kind="ExternalInput").ap()
    k_d = nc.dram_tensor("kernel", [S, KSZ, C], f32, kind="ExternalInput").ap()
    o_d = nc.dram_tensor("out", [BPC, TOUT, KSZ], f32, kind="ExternalOutput").ap()

    with tile.TileContext(nc) as tc, ExitStack() as ctx:
        const = ctx.enter_context(tc.tile_pool(name="const", bufs=1))
        outp = ctx.enter_context(tc.tile_pool(name="outp", bufs=3))
        dram = ctx.enter_context(tc.tile_pool(name="dram", bufs=1, space="DRAM"))

        F_all = const.tile([42, 2048], f16, tag="F_all")
        Hbig = const.tile([42, NSIG * T], f16, tag="Hbig")
        Xnb = const.tile([NSIG, SIGW], f16, tag="Xnb")

        XnD = dram.tile([NSIG, SIGW], f16, tag="XnD")
        OnD = dram.tile([NSIG, 1024], f16, tag="OnD")
        PexpD = dram.tile([NSIG, 4096], f16, tag="PexpD")
        FxD = dram.tile([128, 128], f16, tag="FxD")
        K2D = dram.tile([2, 2048], f16, tag="K2D")

        with tc.tile_pool(name="pprep", bufs=1, space="PSUM") as pprep, \
             tc.tile_pool(name="ldp", bufs=2) as ldp:
            # activation-table preload so the ~1.3us load overlaps input DMAs
            dm0 = ldp.tile([1, 1], f32, tag="dm0")
            nc.vector.memset(dm0[:], 1.0)
            dm1 = ldp.tile([1, 1], f32, tag="dm1")
            nc.scalar.activation(out=dm1[:], in_=dm0[:], func=ACT.Sqrt)

            # input loads first: x on sync, kernel on sync
            X0 = ldp.tile([128, BPC * 32], f32, tag="X0")
            for b in range(BPC):
                nc.sync.dma_start(
                    out=X0[:, b * 32:(b + 1) * 32],
                    in_=_rap(x_d[b:b + 1, 0:1, 0:1],
                             [[8, 128], [1024, 4], [1, 8]]))
            KN = ldp.tile([S, KSZ * C], f32, tag="KN")
            nc.sync.dma_start(out=KN[:], in_=k_d.rearrange("s k c -> s (k c)"))

            ident = const.tile([128, 128], f32, tag="ident")
            make_identity(nc, ident[:])
            identh = const.tile([S, S], f16, tag="identh")
            make_identity(nc, identh[:])
            onesW = ldp.tile([NSIG, 1024], f16, tag="onesW")
            nc.gpsimd.memset(onesW[:], 1.0)
            nc.gpsimd.dma_start(out=OnD[:], in_=onesW[:])

            # ---------- x chain ----------
            PXp = pprep.tile([8, BPC * T], f32, tag="PXp")
            for b in range(BPC):
                for cc in range(4):
                    nc.tensor.transpose(
                        PXp[:, (b * 4 + cc) * 128:(b * 4 + cc + 1) * 128],
                        X0[:, (b * 4 + cc) * 8:(b * 4 + cc + 1) * 8],
                        ident[:, :])
            Xs8 = ldp.tile([8, BPC * T], f32, tag="Xs8")
            nc.scalar.copy(out=Xs8[:], in_=PXp[:])
            Xsig = ldp.tile([NSIG, T], f32, tag="Xsig")
            for b in range(BPC):
                nc.sync.dma_start(
                    out=Xsig[b * C:(b + 1) * C, :],
                    in_=Xs8[:, b * T:(b + 1) * T])

            xst = ldp.tile([NSIG, nc.vector.BN_STATS_DIM], f32, tag="xst")
            nc.vector.bn_stats(out=xst[:], in_=Xsig[:])
            mvx = ldp.tile([NSIG, nc.vector.BN_AGGR_DIM], f32, tag="mvx")
            nc.vector.bn_aggr(out=mvx[:], in_=xst[:])
            xstd = ldp.tile([NSIG, 1], f32, tag="xstd")
            nc.scalar.activation(out=xstd[:], in_=mvx[:, 1:2], func=ACT.Sqrt)
            xrstd = ldp.tile([NSIG, 1], f32, tag="xrstd")
            nc.vector.reciprocal(out=xrstd[:], in_=xstd[:])
            xbias = ldp.tile([NSIG, 1], f32, tag="xbias")
            nc.vector.scalar_tensor_tensor(
                out=xbias[:], in0=mvx[:, 0:1], scalar=-1.0, in1=xrstd[:],
                op0=OP.mult, op1=OP.mult)

            # ---------- shapelet-kernel chain ----------
            kst = ldp.tile([S, nc.vector.BN_STATS_DIM], f32, tag="kst")
            nc.vector.bn_stats(out=kst[:], in_=KN[:])
            mvk = ldp.tile([S, nc.vector.BN_AGGR_DIM], f32, tag="mvk")
            nc.vector.bn_aggr(out=mvk[:], in_=kst[:])
            kstd = ldp.tile([S, 1], f32, tag="kstd")
            nc.scalar.activation(out=kstd[:], in_=mvk[:, 1:2], func=ACT.Sqrt)
            krstd = ldp.tile([S, 1], f32, tag="krstd")
            nc.vector.reciprocal(out=krstd[:], in_=kstd[:])
            kscale = ldp.tile([S, 1], f32, tag="kscale")
            nc.vector.tensor_scalar_mul(kscale[:], krstd[:], -2.0)
            kbias = ldp.tile([S, 1], f32, tag="kbias")
            nc.vector.scalar_tensor_tensor(
                out=kbias[:], in0=mvk[:, 0:1], scalar=2.0, in1=krstd[:],
                op0=OP.mult, op1=OP.mult)
            KNmh = ldp.tile([S, KSZ * C], f16, tag="KNmh")
            nc.vector.tensor_scalar(
                out=KNmh[:], in0=KN[:], scalar1=kscale[:], scalar2=kbias[:],
                op0=OP.mult, op1=OP.add)
            # K2 = 0.25 * sum_c KNmh^2 in fp32 (exact squares of fp16 taps)
            KN2 = ldp.tile([S, KSZ * C], f32, tag="KN2")
            nc.vector.tensor_tensor(
                out=KN2[:], in0=KNmh[:], in1=KNmh[:], op=OP.mult)
            K2sn = ldp.tile([S, KSZ], f32, tag="K2sn")
            nc.vector.tensor_reduce(
                out=K2sn[:], in_=KN2[:].rearrange("s (k c) -> s k c", c=C),
                axis=AX.X, op=OP.add)
            nc.vector.tensor_scalar_mul(K2sn[:], K2sn[:], 0.25)
            K2hi = ldp.tile([S, KSZ], f16, tag="K2hi")
            nc.scalar.copy(out=K2hi[:], in_=K2sn[:])
            K2hi32 = ldp.tile([S, KSZ], f32, tag="K2hi32")
            nc.scalar.copy(out=K2hi32[:], in_=K2hi[:])
            K2lo = ldp.tile([S, KSZ], f16, tag="K2lo")
            nc.vector.tensor_tensor(
                out=K2lo[:], in0=K2sn[:], in1=K2hi32[:], op=OP.subtract)

            # transposes + staging of taps / K2
            TPp = pprep.tile([128, 128], f16, tag="TPp")
            nc.tensor.transpose(TPp[:, 0:64], KNmh[:, 0:128], identh[:, :])
            nc.tensor.transpose(TPp[:, 64:128], KNmh[:, 128:256], identh[:, :])
            K2Tph = pprep.tile([KSZ, S], f16, tag="K2Tph")
            nc.tensor.transpose(K2Tph[:], K2hi[:], identh[:, :])
            K2Tpl = pprep.tile([KSZ, S], f16, tag="K2Tpl")
            nc.tensor.transpose(K2Tpl[:], K2lo[:], identh[:, :])
            TPsb = ldp.tile([128, 128], f16, tag="TPsb")
            nc.scalar.copy(out=TPsb[:], in_=TPp[:])
            K2sb = ldp.tile([KSZ, 128], f16, tag="K2sb")
            nc.scalar.copy(out=K2sb[:, 0:64], in_=K2Tph[:])
            nc.scalar.copy(out=K2sb[:, 64:128], in_=K2Tpl[:])
            # FxD[kp*512 + c*64 + s] = KNmh[s, kp*8+c]
            nc.scalar.dma_start(
                out=_rap(FxD[0:1, 0:1], [[512, 16], [64, 8], [1, 64]]),
                in_=TPsb[:, 0:64])
            nc.scalar.dma_start(
                out=_rap(FxD[64:65, 0:1], [[512, 16], [64, 8], [1, 64]]),
                in_=TPsb[:, 64:128])
            nc.scalar.dma_start(
                out=_rap(K2D[0:1, 0:1], [[64, 32], [2048, 2], [1, 64]]),
                in_=_rap(K2sb[0:1, 0:1], [[128, 32], [64, 2], [1, 64]]))

            # ---------- F_all scatter ----------
            nc.gpsimd.memset(F_all[:], 0.0)
            for j in range(4):
                nc.sync.dma_start(
                    out=_rap(F_all[8 * j:8 * j + 8, 64 * j:64 * j + 1],
                             [[2048, 8], [256, 8], [1, 64]]),
                    in_=_rap(FxD[4 * j:4 * j + 1, 0:1],
                             [[64, 8], [2048, 8], [1, 64]]))
            # delta_j window-power taps: rows 32+j (hi) / 36+j (lo), block j
            for base in (32, 36):
                for j in range(4):
                    eng = nc.scalar if base == 32 else nc.gpsimd
                    eng.dma_start(
                        out=_rap(F_all[base + j:base + j + 1, 64 * j:64 * j + 1],
                                 [[2048, 1], [256, 8], [1, 64]]),
                        in_=_rap(OnD[0:1, 0:1], [[64, 8], [1, 64]]))
            nc.scalar.dma_start(out=F_all[40:42, :], in_=K2D[:])

            # x-hat (fp16) + zero pad; stage for the Hankel loads
            nc.vector.memset(Xnb[:, 512:SIGW], 0.0)
            nc.vector.tensor_scalar(
                out=Xnb[:, 0:T], in0=Xsig[:], scalar1=xrstd[:], scalar2=xbias[:],
                op0=OP.mult, op1=OP.add)
            nc.sync.dma_start(
                out=_rap(XnD[0:1, 0:1], [[SIGW, NSIG], [1, SIGW]]),
                in_=Xnb[:, :])

            # ---------- P-hat path ----------
            # x2f = x-hat^2 (exact fp32), sliding window-8 sums
            x2f = ldp.tile([NSIG, SIGW], f32, tag="x2f")
            nc.vector.tensor_tensor(
                out=x2f[:], in0=Xnb[:, :], in1=Xnb[:, :], op=OP.mult)
            Pw1 = ldp.tile([NSIG, SIGW], f32, tag="Pw1")
            nc.vector.tensor_tensor(
                out=Pw1[:, 0:543], in0=x2f[:, 0:543], in1=x2f[:, 1:544],
                op=OP.add)
            Pw2 = ldp.tile([NSIG, SIGW], f32, tag="Pw2")
            nc.vector.tensor_tensor(
                out=Pw2[:, 0:541], in0=Pw1[:, 0:541], in1=Pw1[:, 2:543],
                op=OP.add)
            Pw = ldp.tile([NSIG, SIGW], f32, tag="Pw")
            nc.vector.memset(Pw[:, 536:SIGW], 0.0)
            nc.vector.tensor_tensor(
                out=Pw[:, 0:537], in0=Pw2[:, 0:537], in1=Pw2[:, 4:541],
                op=OP.add)
            # split P into fp16 hi + lo
            Phi = ldp.tile([NSIG, SIGW], f16, tag="Phi")
            nc.scalar.copy(out=Phi[:], in_=Pw[:])
            Phi32 = ldp.tile([NSIG, SIGW], f32, tag="Phi32")
            nc.scalar.copy(out=Phi32[:], in_=Phi[:])
            Plo = ldp.tile([NSIG, SIGW], f16, tag="Plo")
            nc.vector.tensor_tensor(
                out=Plo[:], in0=Pw[:], in1=Phi32[:], op=OP.subtract)
            # PwHL[sig, (h2, j, t)] = P_{hi/lo}[sig][8j + t]
            PwHL = ldp.tile([NSIG, 4096], f16, tag="PwHL")
            nc.sync.dma_start(
                out=PwHL[:, 0:2048],
                in_=_rap(Phi[0:1, 0:1], [[SIGW, NSIG], [8, 4], [1, 512]]))
            nc.sync.dma_start(
                out=PwHL[:, 2048:4096],
                in_=_rap(Plo[0:1, 0:1], [[SIGW, NSIG], [8, 4], [1, 512]]))
            nc.sync.dma_start(
                out=_rap(PexpD[0:1, 0:1], [[4096, NSIG], [1, 4096]]),
                in_=PwHL[:, :])

            # ---------- Hankel loads ----------
            # P rows 32..39 for all signals in one DMA
            nc.sync.dma_start(
                out=Hbig[32:40, :],
                in_=_rap(PexpD[0:1, 0:1], [[512, 8], [4096, NSIG], [1, 512]]))
            # ones rows 40..41
            nc.gpsimd.dma_start(
                out=Hbig[40:42, :],
                in_=_rap(OnD[0:1, 0:1], [[8192, 2], [1, 8192]]))
            # xn rows 0..31, two signals per DMA
            for i in range(8):
                eng = nc.sync if i % 2 == 0 else nc.scalar
                eng.dma_start(
                    out=_rap(Hbig[0:1, 2 * i * T:2 * i * T + 1],
                             [[NSIG * T, KSZ], [T, 2], [1, T]]),
                    in_=_rap(XnD[2 * i:2 * i + 1, 0:1],
                             [[1, KSZ], [SIGW, 2], [1, T]]))

        # ---------- main loop: matmuls + min + store ----------
        with tc.tile_pool(name="pmm", bufs=2, space="PSUM") as pmm:
            for b in range(BPC):
                for cc in range(4):
                    c0 = cc * 128
                    cnt = 128 if cc < 3 else TOUT - 3 * 128
                    acc = pmm.tile([128, 2048], f32, tag="acc")
                    for ch in range(C):
                        sig = b * C + ch
                        nc.tensor.matmul(
                            acc[:, ch * 256:(ch + 1) * 256],
                            lhsT=Hbig[0:42, sig * T + c0:sig * T + c0 + 128],
                            rhs=F_all[0:42, ch * 256:(ch + 1) * 256],
                            start=True, stop=True)
                    PM = outp.tile([128, KSZ], f32, tag="PM")
                    nc.vector.tensor_reduce(
                        out=PM[:],
                        in_=acc[:].rearrange("p (g s) -> p g s", s=S),
                        axis=AX.X, op=OP.min)
                    nc.gpsimd.dma_start(out=o_d[b, c0:c0 + cnt, :],
                                        in_=PM[0:cnt, :])

    nc.compile()
    return nc


def get_nc():
    if "nc" not in _cache:
        _cache["nc"] = _build_nc()
    return _cache["nc"]


def kernel(x: np.ndarray, kernel: np.ndarray) -> np.ndarray:
    from concourse.bass_utils import run_bass_kernel_spmd

    nc = get_nc()
    x = np.ascontiguousarray(x, dtype=np.float32)
    kern = np.ascontiguousarray(kernel, dtype=np.float32)
    in_maps = [
        {"x": x[i * BPC:(i + 1) * BPC], "kernel": kern} for i in range(NCORES)
    ]
    res = run_bass_kernel_spmd(nc, in_maps, core_ids=list(range(NCORES)))
    return np.concatenate([r["out"] for r in res.results], axis=0)


if __name__ == "__main__":
    rng = np.random.default_rng(0)
    x = rng.standard_normal((B, T, C), dtype=np.float32)
    k = rng.uniform(-0.05, 0.05, (S, KSZ, C)).astype(np.float32)
    out = kernel(x=x, kernel=k)
    print(out.shape, out.dtype)
